# revision 1
# baseline (speedup 1.0000x reference)
"""Trainium2 Bass kernel for nn_Autoencoder__gen204 (8-core data parallel).

Network: enc(2048->128->64->32 relu MLP) -> 4-qubit statevector circuit on
latent[:, :4] -> dec(4->64->128->2048 relu MLP).

Key transform: the quantum circuit is RX-encoding (per-sample angles) followed
by a FIXED unitary V(qw). E_q = Tr(rho . V^dag Z_q V) with rho a product state
whose per-qubit Bloch vectors are (0, -sin t, cos t). Expanding the fixed
observable in the Pauli basis, only {I,Y,Z}^4 strings survive, so
refined = phi @ W81 where phi is the 81-dim tensor product of per-qubit
features [1, sin t, cos t] and W81 is an (81,4) matrix computed on host from
qw (Y-string signs folded in). W81 @ dec_w0 then folds into the decoder, so
the whole model is matmuls + relu + sin/cos + 3 elementwise products.

Device layout: feature-major (features on SBUF partitions, batch on the free
dim) so every matmul contraction dim sits on partitions. Host transposes x to
(D, B) and transposes the output back; per-core batch shard is 2048 columns.
Matmuls run in bf16 (weights host-cast; x cast f32->bf16 inline by the
SWDGE DMA engines); PSUM accumulation is fp32; the output is written as
fp16 (|out| <= ~1e-3, so fp16 keeps ~2.4e-4 relative precision) and
converted back to fp32 on the host. Emission is software-pipelined:
L1(n) is interleaved with mid(n-1) and decoder(n-2) work so the PE
always has ready instructions, with dummy-matmul bursts to keep the PE
clock gate (HAM) warm.
"""

import ml_dtypes
import numpy as np

import concourse.bass as bass
import concourse.mybir as mybir
import concourse.tile as tile
from concourse import bacc
from concourse.bass_utils import run_bass_kernel_spmd

# ----- problem constants (hardcoded per contract) -----
B, D, H1, H2, L = 16384, 2048, 128, 64, 32
NQ, NL = 4, 3
NCORES = 8
BL = B // NCORES  # 2048 batch per core
P = 128
KD = D // P  # 16 k-chunks for the D contraction
HALF = BL // 2  # 1024: batch processed in two halves per core
F32 = mybir.dt.float32
F32R = mybir.dt.float32r
BF16 = mybir.dt.bfloat16
F16 = mybir.dt.float16

# =====================================================================
# Host-side quantum-circuit collapse: qw -> W81 (81, 4)
# =====================================================================

_I2 = np.eye(2, dtype=np.complex128)
_SY = np.array([[0, -1j], [1j, 0]], dtype=np.complex128)
_SZ = np.array([[1, 0], [0, -1]], dtype=np.complex128)
_CNOT4 = np.array(
    [[1, 0, 0, 0], [0, 1, 0, 0], [0, 0, 0, 1], [0, 0, 1, 0]], dtype=np.complex128
).reshape(2, 2, 2, 2)
_bits = (np.arange(2**NQ)[:, None] >> np.arange(NQ - 1, -1, -1)) & 1
_Z_SIGNS = (1 - 2 * _bits).astype(np.float64)  # (16, 4)


def _rot_mat(phi, theta, omega):
    ez = np.exp(-0.5j * phi)
    rz1 = np.array([[ez, 0], [0, np.conj(ez)]], dtype=np.complex128)
    c, s = np.cos(theta / 2), np.sin(theta / 2)
    ry = np.array([[c, -s], [s, c]], dtype=np.complex128)
    eo = np.exp(-0.5j * omega)
    rz2 = np.array([[eo, 0], [0, np.conj(eo)]], dtype=np.complex128)
    return rz2 @ ry @ rz1


def _apply1(state, U, wire):
    state = np.tensordot(U, state, axes=[[1], [wire]])
    return np.moveaxis(state, 0, wire)


def _apply_cnot(state, c, t):
    state = np.tensordot(_CNOT4, state, axes=[[2, 3], [c, t]])
    return np.moveaxis(state, [0, 1], [c, t])


def _w81_from_qw(qw):
    qw = np.asarray(qw, dtype=np.float64)
    V = np.eye(16, dtype=np.complex128).reshape(2, 2, 2, 2, 16)
    for layer in range(NL):
        for q in range(NQ):
            V = _apply1(V, _rot_mat(*qw[layer, q]), q)
        for q in range(NQ - 1):
            V = _apply_cnot(V, q, q + 1)
    V = V.reshape(16, 16)
    paulis = [_I2, _SY, _SZ]  # digit 0 -> I(1), 1 -> Y(sin), 2 -> Z(cos)
    W = np.zeros((81, NQ), dtype=np.float64)
    for q in range(NQ):
        O = V.conj().T @ (_Z_SIGNS[:, q][:, None] * V)
        for k in range(81):
            d = [(k // 27) % 3, (k // 9) % 3, (k // 3) % 3, k % 3]
            Pm = np.kron(
                np.kron(paulis[d[0]], paulis[d[1]]),
                np.kron(paulis[d[2]], paulis[d[3]]),
            )
            alpha = np.trace(O @ Pm) / 16.0
            n_y = sum(1 for x in d if x == 1)
            W[k, q] = alpha.real * ((-1) ** n_y)  # t(Y) = -sin; feature is +sin
    return W


def _selection_matrices():
    """(9, 324) f32; column block q*81..(q+1)*81 is Sel_q mapping the 9-row
    feature stack [1, sin t0..3, cos t0..3] to the 81 phi rows."""
    S = np.zeros((9, 4 * 81), dtype=np.float32)
    for q in range(NQ):
        for k in range(81):
            d = (k // (3 ** (3 - q))) % 3
            row = 0 if d == 0 else (1 + q if d == 1 else 5 + q)
            S[row, q * 81 + k] = 1.0
    return S


# =====================================================================
# Bass program (one core; identical across the 8 cores)
# =====================================================================

_PROGRAM_CACHE = {}


def _build_program(debug=False, dec2_bias_zero=False, paired_evict=False,
                   ps_cfg=(1, 3, 4), warmup=40, warmup2=12, warmup3=0,
                   xsplit="hh", wbufs=2, sbufs=6):
    key = (debug, dec2_bias_zero, paired_evict, ps_cfg, warmup, warmup2, warmup3, xsplit, wbufs, sbufs)
    if key in _PROGRAM_CACHE:
        return _PROGRAM_CACHE[key]

    nc = bacc.Bacc("TRN2", target_bir_lowering=False, debug=debug)

    xt = nc.dram_tensor("xt", [D, BL], F32, kind="ExternalInput")
    w0p = nc.dram_tensor("w0p", [P, D], BF16, kind="ExternalInput")
    w1 = nc.dram_tensor("w1", [H1, H2], BF16, kind="ExternalInput")
    # w2a = [0 | enc_w2[:, :4] | enc_w2[:, :4]]: the L3 matmul directly
    # produces the 9-row pre-activation whose Sin (with bias
    # [pi/2, b2, b2 + pi/2]) is the feature stack [1, sin t, cos t].
    w2a = nc.dram_tensor("w2a", [H2, 9], BF16, kind="ExternalInput")
    selc = nc.dram_tensor("selc", [9, 324], BF16, kind="ExternalInput")
    wf = nc.dram_tensor("wf", [81, H2], BF16, kind="ExternalInput")
    wd1 = nc.dram_tensor("wd1", [H2, H1], BF16, kind="ExternalInput")
    wd2 = nc.dram_tensor("wd2", [H1, D], BF16, kind="ExternalInput")
    bia = nc.dram_tensor("bia", [P, 21], F32, kind="ExternalInput")
    # fp16 output: |out| <= ~1e-3 so fp16 keeps ~2.4e-4 relative precision
    # while halving output DMA bytes; host converts back to f32.
    outt = nc.dram_tensor("outt", [D, BL], F16, kind="ExternalOutput")

    Relu = mybir.ActivationFunctionType.Relu
    Sin = mybir.ActivationFunctionType.Sin

    with tile.TileContext(nc) as tc:
        with (
            tc.tile_pool(name="const", bufs=1) as cpool,
            tc.tile_pool(name="xin", bufs=16) as xpool,
            tc.tile_pool(name="work", bufs=wbufs) as wpool,
            tc.tile_pool(name="stage", bufs=sbufs) as spool,
            tc.tile_pool(name="ps1p", bufs=ps_cfg[0], space="PSUM") as ps1pool,
            tc.tile_pool(name="psmid", bufs=ps_cfg[1], space="PSUM") as psmpool,
            tc.tile_pool(name="psout", bufs=ps_cfg[2], space="PSUM") as psopool,
        ):
            # ---- x in: 64 cast-DMAs of [128, 512] in batch-chunk-major
            # order, issued first: chunk n's tiles all land ~(n+1)/4 of the
            # way through the stream, so each chunk's L1 finishes on time and
            # only the last chunk's pipeline drains after the x stream ends.
            # (SWDGE casts f32 -> bf16 inline.) ----
            # xsplit "hh": two 1024-col groups (32 DMAs of 512KB).
            # xsplit "211": 1024 + 512 + 512 cols (48 DMAs) — chunk 2's
            # tiles land ~3/4 through the stream so only chunk 3's pipeline
            # drains after x-in ends.
            if xsplit == "hh":
                groups = [(0, HALF), (HALF, HALF)]
            else:
                groups = [(0, HALF), (HALF, 512), (HALF + 512, 512)]
            xts = {}
            for gi, (gc0, gw) in enumerate(groups):
                for k in range(KD):
                    t = xpool.tile([P, gw], BF16, name=f"x_{gi}_{k}", tag=f"xt{gi}")
                    nc.gpsimd.dma_start(
                        out=t[:],
                        in_=xt[k * P : (k + 1) * P, gc0 : gc0 + gw],
                    )
                    xts[(gi, k)] = t

            def x_slice(n, k):
                c = n * 512
                for gi, (gc0, gw) in enumerate(groups):
                    if gc0 <= c < gc0 + gw:
                        return xts[(gi, k)][:, c - gc0 : c - gc0 + 512]
                raise AssertionError(n)

            # ---- resident constants ----
            w0p_sb = cpool.tile([P, D], BF16, name="w0p_sb", tag="w0p_sb")
            nc.sync.dma_start(out=w0p_sb[:], in_=w0p[:])
            w1_sb = cpool.tile([H1, H2], BF16, name="w1_sb", tag="w1_sb")
            nc.sync.dma_start(out=w1_sb[:], in_=w1[:])
            w2a_sb = cpool.tile([H2, 9], BF16, name="w2a_sb", tag="w2a_sb")
            nc.sync.dma_start(out=w2a_sb[:], in_=w2a[:])
            selc_sb = cpool.tile([9, 324], BF16, name="selc_sb", tag="selc_sb")
            nc.sync.dma_start(out=selc_sb[:], in_=selc[:])
            wf_sb = cpool.tile([81, H2], BF16, name="wf_sb", tag="wf_sb")
            nc.sync.dma_start(out=wf_sb[:], in_=wf[:])
            wd1_sb = cpool.tile([H2, H1], BF16, name="wd1_sb", tag="wd1_sb")
            nc.sync.dma_start(out=wd1_sb[:], in_=wd1[:])
            wd2_sb = cpool.tile([H1, D], BF16, name="wd2_sb", tag="wd2_sb")
            nc.sync.dma_start(out=wd2_sb[:], in_=wd2[:])
            bia_sb = cpool.tile([P, 21], F32, name="bia_sb", tag="bia_sb")
            nc.sync.dma_start(out=bia_sb[:], in_=bia[:])

            state = {}

            def emit_l1(n):
                """16 accumulating matmuls + relu eviction for batch chunk n."""
                ps1 = ps1pool.tile([P, 512], F32, name=f"ps1_{n}", tag="ps1")
                h1 = wpool.tile([P, 512], BF16, name=f"h1_{n}", tag="h1")
                state[n] = h1
                steps = []
                for k in range(KD):
                    def mm(k=k, ps1=ps1, n=n):
                        nc.tensor.matmul(
                            ps1[:],
                            w0p_sb[:, k * P : (k + 1) * P],
                            x_slice(n, k),
                            start=(k == 0),
                            stop=(k == KD - 1),
                        )
                    steps.append(mm)
                def ev(ps1=ps1, h1=h1):
                    nc.scalar.activation(h1[:], ps1[:], Relu, bias=bia_sb[:, 0:1])
                steps.append(ev)
                return steps

            def emit_mid(n):
                """Mid layers for batch chunk n (produces h4)."""
                h1 = state[n]
                steps = []

                ps2 = psmpool.tile([H2, 512], F32, name=f"ps2_{n}", tag="psm")
                h2 = wpool.tile([H2, 512], BF16, name=f"h2_{n}", tag="h2")
                steps.append(lambda: nc.tensor.matmul(ps2[:], w1_sb[:], h1[:], start=True, stop=True))
                steps.append(lambda: nc.scalar.activation(h2[:], ps2[:], Relu, bias=bia_sb[0:H2, 1:2]))

                ps3 = psmpool.tile([9, 512], F32, name=f"ps3_{n}", tag="psm")
                mst = wpool.tile([9, 512], BF16, name=f"mst_{n}", tag="mst")
                steps.append(lambda: nc.tensor.matmul(ps3[:], w2a_sb[:], h2[:], start=True, stop=True))
                steps.append(lambda: nc.scalar.activation(mst[:], ps3[:], Sin, bias=bia_sb[0:9, 2:3]))

                psA0 = psmpool.tile([81, 512], F32, name=f"psA0_{n}", tag="psm")
                psA1 = psmpool.tile([81, 512], F32, name=f"psA1_{n}", tag="psm")
                s1 = wpool.tile([81, 512], F32, name=f"s1_{n}", tag="s1")
                t01 = wpool.tile([81, 512], F32, name=f"t01_{n}", tag="t01")
                steps.append(lambda: nc.tensor.matmul(psA0[:], selc_sb[:, 0:81], mst[:], start=True, stop=True))
                steps.append(lambda: nc.tensor.matmul(psA1[:], selc_sb[:, 81:162], mst[:], start=True, stop=True))
                steps.append(lambda: nc.vector.tensor_copy(s1[:], psA1[:]))
                steps.append(lambda: nc.vector.tensor_mul(t01[:], psA0[:], s1[:]))
                psA2 = psmpool.tile([81, 512], F32, name=f"psA2_{n}", tag="psm")
                psA3 = psmpool.tile([81, 512], F32, name=f"psA3_{n}", tag="psm")
                s3 = wpool.tile([81, 512], F32, name=f"s3_{n}", tag="s3")
                t23 = wpool.tile([81, 512], F32, name=f"t23_{n}", tag="t23")
                phi = wpool.tile([81, 512], BF16, name=f"phi_{n}", tag="phi")
                steps.append(lambda: nc.tensor.matmul(psA2[:], selc_sb[:, 162:243], mst[:], start=True, stop=True))
                steps.append(lambda: nc.tensor.matmul(psA3[:], selc_sb[:, 243:324], mst[:], start=True, stop=True))
                steps.append(lambda: nc.scalar.copy(s3[:], psA3[:]))
                steps.append(lambda: nc.vector.tensor_mul(t23[:], psA2[:], s3[:]))
                steps.append(lambda: nc.vector.tensor_mul(phi[:], t01[:], t23[:]))

                ps4 = psmpool.tile([H2, 512], F32, name=f"ps4_{n}", tag="psm")
                h3 = wpool.tile([H2, 512], BF16, name=f"h3_{n}", tag="h3")
                steps.append(lambda: nc.tensor.matmul(ps4[:], wf_sb[:], phi[:], start=True, stop=True))
                steps.append(lambda: nc.scalar.activation(h3[:], ps4[:], Relu, bias=bia_sb[0:H2, 3:4]))

                ps5 = psmpool.tile([H1, 512], F32, name=f"ps5_{n}", tag="psm")
                h4 = wpool.tile([H1, 512], BF16, name=f"h4_{n}", tag="h4")
                steps.append(lambda: nc.tensor.matmul(ps5[:], wd1_sb[:], h3[:], start=True, stop=True))
                steps.append(lambda: nc.scalar.activation(h4[:], ps5[:], Relu, bias=bia_sb[0:H1, 4:5]))
                state[("h4", n)] = h4
                return steps

            def emit_dec(n):
                """Decoder head + out-DMAs for batch chunk n."""
                h4 = state[("h4", n)]
                c0 = n * 512
                steps = []
                # two mo blocks can share one 2-bank PSUM tile so one wide
                # eviction drains both (evictions gate the MM stream)
                PW = 2 if paired_evict else 1
                for mo in range(0, KD, 2):
                    for g in range(2 // PW):
                        mg = mo + g * PW
                        ps6 = psopool.tile([P, 512 * PW], F32, name=f"ps6_{n}_{mg}", tag="pso")
                        ost = spool.tile([P, 512 * PW], F16, name=f"ost_{n}_{mg}", tag="ost")
                        for j in range(PW):
                            def mm6(ps6=ps6, mg=mg, j=j, h4=h4):
                                nc.tensor.matmul(
                                    ps6[:, j * 512 : (j + 1) * 512],
                                    wd2_sb[:, (mg + j) * P : (mg + j + 1) * P],
                                    h4[:],
                                    start=True,
                                    stop=True,
                                )
                            steps.append(mm6)
                        use_dve = (mg + n) % 2 == 0
                        if dec2_bias_zero:
                            if use_dve:
                                def ev6(ost=ost, ps6=ps6):
                                    nc.vector.tensor_copy(ost[:], ps6[:])
                            else:
                                def ev6(ost=ost, ps6=ps6):
                                    nc.scalar.copy(ost[:], ps6[:])
                            steps.append(ev6)
                        else:
                            for j in range(PW):
                                if use_dve:
                                    def ev6(ost=ost, ps6=ps6, mg=mg, j=j):
                                        nc.vector.tensor_scalar_add(
                                            ost[:, j * 512 : (j + 1) * 512],
                                            ps6[:, j * 512 : (j + 1) * 512],
                                            bia_sb[:, 5 + mg + j : 6 + mg + j],
                                        )
                                else:
                                    def ev6(ost=ost, ps6=ps6, mg=mg, j=j):
                                        nc.scalar.add(
                                            ost[:, j * 512 : (j + 1) * 512],
                                            ps6[:, j * 512 : (j + 1) * 512],
                                            bia_sb[:, 5 + mg + j : 6 + mg + j],
                                        )
                                steps.append(ev6)
                        for j in range(PW):
                            # late chunks: gpsimd (SWDGE) is idle once x is in,
                            # so split out-DMA issue across both DGE paths
                            use_gp = n >= 2 and (mg + j) % 2 == 1
                            def dma6(ost=ost, mg=mg, j=j, c0=c0, use_gp=use_gp):
                                eng = nc.gpsimd if use_gp else nc.sync
                                eng.dma_start(
                                    out=outt[(mg + j) * P : (mg + j + 1) * P, c0 : c0 + 512],
                                    in_=ost[:, j * 512 : (j + 1) * 512],
                                )
                            steps.append(dma6)
                return steps

            def ratio_merge(*streams):
                streams = [s for s in streams if s]
                out = []
                idx = [0] * len(streams)
                total = sum(len(s) for s in streams)
                for _ in range(total):
                    # advance the stream with the least relative progress
                    best, bestv = None, None
                    for si, s in enumerate(streams):
                        if idx[si] >= len(s):
                            continue
                        v = idx[si] / len(s)
                        if bestv is None or v < bestv:
                            best, bestv = si, v
                    out.append(streams[best][idx[best]])
                    idx[best] += 1
                return out

            # preload the Sin ACT table set while the Scalar engine is idle
            # (otherwise the first Sin at ~40us swaps tables mid-kernel,
            # serializing ~2.6us of eviction work)
            dsin = cpool.tile([1, 8], F32, name="dsin", tag="dsin")
            nc.vector.memset(dsin[:], 0.0)
            nc.scalar.activation(dsin[:, 4:8], dsin[:, 0:4], Sin)

            # PE warm-up: dummy matmuls fill the PE while the first x tiles
            # stream in, so HAM un-throttles before L1 starts
            if warmup:
                wtile = cpool.tile([P, 512], BF16, name="wtile", tag="wtile")
                nc.vector.memset(wtile[:], 0.0)
                wps = psopool.tile([P, 512], F32, name="wps", tag="pso")
                for i in range(warmup):
                    nc.tensor.matmul(
                        wps[:], wtile[:, 0:128], wtile[:], start=(i == 0),
                        stop=(i == warmup - 1),
                    )

            # software pipeline: L1(n) interleaved with mid(n-1) and dec(n-2)
            # so PE always has dense ready work through the tail
            for n in range(6):
                if n == 5 and warmup and warmup3:
                    wps3 = ps1pool.tile([P, 512], F32, name="wps3", tag="ps1")
                    for i in range(warmup3):
                        nc.tensor.matmul(
                            wps3[:], wtile[:, 0:128], wtile[:], start=(i == 0),
                            stop=(i == warmup3 - 1),
                        )
                if n == 4 and warmup and warmup2:
                    # re-warm the PE clock gate before the tail phase
                    wps2 = ps1pool.tile([P, 512], F32, name="wps2", tag="ps1")
                    for i in range(warmup2):
                        nc.tensor.matmul(
                            wps2[:], wtile[:, 0:128], wtile[:], start=(i == 0),
                            stop=(i == warmup2 - 1),
                        )
                a = emit_l1(n) if n < 4 else []
                b = emit_mid(n - 1) if 1 <= n <= 4 else []
                c = emit_dec(n - 2) if n >= 2 else []
                for step in ratio_merge(a, b, c):
                    step()

    nc.compile()
    _PROGRAM_CACHE[key] = nc
    return nc


# =====================================================================
# Host wrapper: shard, run, gather
# =====================================================================


def make_in_maps(
    x, enc_w0, enc_b0, enc_w1, enc_b1, enc_w2, enc_b2, qw,
    dec_w0, dec_b0, dec_w1, dec_b1, dec_w2, dec_b2,
):
    f32 = np.float32
    # W81 from the circuit, folded into the decoder's first layer
    w81 = _w81_from_qw(np.asarray(qw, dtype=np.float64))
    wfold = (w81 @ np.asarray(dec_w0, dtype=np.float64)).astype(f32)  # (81, 64)

    # enc_w0 repacked so SBUF col-block k holds rows k*128..(k+1)*128
    w0p = (
        np.asarray(enc_w0, f32).reshape(KD, P, H1).transpose(1, 0, 2).reshape(P, D)
    )
    w0p = np.ascontiguousarray(w0p)

    b2q = np.asarray(enc_b2, f32)[:NQ]
    pi2 = np.float32(np.pi / 2)
    bia = np.zeros((P, 21), dtype=f32)
    bia[:, 0] = enc_b0
    bia[:H2, 1] = enc_b1
    bia[0, 2] = pi2  # row 0 of feature stack: sin(pi/2) = 1
    bia[1 : 1 + NQ, 2] = b2q  # sin(t)
    bia[5 : 5 + NQ, 2] = b2q + pi2  # cos(t)
    bia[:H2, 3] = dec_b0
    bia[:H1, 4] = dec_b1
    bia[:, 5 : 5 + KD] = np.asarray(dec_b2, f32).reshape(KD, P).T

    w2q = np.asarray(enc_w2, f32)[:, :NQ]
    w2a = np.concatenate([np.zeros((H2, 1), f32), w2q, w2q], axis=1)

    bf16 = ml_dtypes.bfloat16
    common = {
        "w0p": w0p.astype(bf16),
        "w1": np.ascontiguousarray(np.asarray(enc_w1, f32)).astype(bf16),
        "w2a": np.ascontiguousarray(w2a).astype(bf16),
        "selc": _selection_matrices().astype(bf16),
        "wf": np.ascontiguousarray(wfold).astype(bf16),
        "wd1": np.ascontiguousarray(np.asarray(dec_w1, f32)).astype(bf16),
        "wd2": np.ascontiguousarray(np.asarray(dec_w2, f32)).astype(bf16),
        "bia": bia,
    }

    xtf = np.ascontiguousarray(np.asarray(x, f32).T)  # (D, B)
    in_maps = []
    for c in range(NCORES):
        m = dict(common)
        m["xt"] = np.ascontiguousarray(xtf[:, c * BL : (c + 1) * BL])
        in_maps.append(m)
    return in_maps


def gather_output(results):
    outt = np.concatenate([results[c]["outt"] for c in range(NCORES)], axis=1)
    return np.ascontiguousarray(outt.T).astype(np.float32)  # (B, D)


def kernel(**inputs):
    nc = _build_program(
        dec2_bias_zero=not np.any(np.asarray(inputs["dec_b2"], np.float32))
    )
    in_maps = make_in_maps(**inputs)
    res = run_bass_kernel_spmd(nc, in_maps, core_ids=list(range(NCORES)))
    return gather_output(res.results)


if __name__ == "__main__":
    # quick self-exercise with random inputs (no reference available here)
    rng = np.random.default_rng(0)
    demo = {
        "x": rng.normal(size=(B, D)).astype(np.float32),
        "enc_w0": rng.normal(size=(D, H1)).astype(np.float32) * 0.02,
        "enc_b0": np.zeros(H1, np.float32),
        "enc_w1": rng.normal(size=(H1, H2)).astype(np.float32) * 0.02,
        "enc_b1": np.zeros(H2, np.float32),
        "enc_w2": rng.normal(size=(H2, L)).astype(np.float32) * 0.02,
        "enc_b2": np.zeros(L, np.float32),
        "qw": rng.normal(size=(NL, NQ, 3)).astype(np.float32),
        "dec_w0": rng.normal(size=(NQ, H2)).astype(np.float32) * 0.02,
        "dec_b0": np.zeros(H2, np.float32),
        "dec_w1": rng.normal(size=(H2, H1)).astype(np.float32) * 0.02,
        "dec_b1": np.zeros(H1, np.float32),
        "dec_w2": rng.normal(size=(H1, D)).astype(np.float32) * 0.02,
        "dec_b2": np.zeros(D, np.float32),
    }
    out = kernel(**demo)
    print("kernel ran, out shape:", out.shape, "finite:", np.isfinite(out).all())

